# revision 66
# baseline (speedup 1.0000x reference)
"""Trainium2 Bass kernel for nn_Attention_86268713108190.

7 independent attention "bands" over batch 8, n=512, d=512, 8 heads,
shared Wqkv/Wout. Sharding: data-parallel over batch — core c handles
batch index c (7 band-samples of [512, 512] each).

Per-core dataflow (per band; all matmuls in float32r, which streams
1 row/cycle at ~2.4 GHz for moving size >= 256, same as bf16; PE
matmul time = out-free-size rows, independent of K and M):
  1. qT/kT = Wqkv @ x^T  (lhsT = WqkvT chunks, rhs = x^T)   [e, n]
  2. v    = x @ Wv^T     (lhsT = x^T chunks,   rhs = WvT)   [n, ev]
     v_aug: per head 64 v-cols + a ones column (65) -> softmax
     denominator falls out of the AV matmul for free
  3. per head pair: S^T = k_h q_h^T (two K=64 matmuls packed into one
     2-bank PSUM tile via tile_position), ONE expS^T = exp(SCALE*S^T)
     activation per jt ([128, 1024] — halving ACT op count measurably
     beats separate [128, 512] exps; no max-subtraction needed:
     |SCALE*S| <~ 1.1 for this distribution), O_aug^T[65, n] =
     v_aug.T @ expS^T accumulated over j-tiles; row 64 = softmax
     denominator. Per pair, DVE copies the RAW denominator rows into a
     parity-major [1, 2, 4, n] tile and the unnormalized O^T to SBUF
     (frees PSUM fast; keeps ACT's in-order exp stream untouched —
     putting Ln/reciprocal ops there measured +35-45 us).
  4. batched tail, lagged one full band: one DRAM bounce, then 8
     stride-0 broadcast DMAs materialize raw d across [128, 4, n]
     (kept as 8 SEPARATE DMAs on purpose — one big batched DMA
     serializes on a single DMA engine/queue, ~25 us slower; SBUF
     stride-0 APs are illegal and GpSimd partition_broadcast cannot
     write at a nonzero partition offset on HW; DRAM-source stride-0
     DMAs lower fine and DMA writes any partition range). 1/d =
     exp(-ln d) is then computed PARTITION-PARALLEL on the broadcast
     tile: two [128, 2048] ACT ops during the (ACT-idle) projection
     phase replace 8 single-lane Lns + a [1, 4096] single-lane exp
     in the exp stream (-45 us; DVE reciprocal is ~1.5-2 us/op on HW,
     far above its cost model — unusable in any shape). One
     [128, 4, n] DVE multiply normalizes O^T.
  5. out = O @ Wout^T + bias for the lagged band, batched store.

Schedule per band s: [bounce DMAs + broadcast exp(-ln) for band s-1
issued first] 8 qk groups, [normalize mul for s-1 on DVE, broadcast
landed by now], 4 v groups, [output projection of s-1], 4 attention
pairs.
Measured dead ends on this HW: fine-grained interleaving of band s+1
projection groups into band s's attention pairs (cross-engine
interlock overhead beats the exp-wait fill: +40-75 us), PE mask-matmul
broadcast of 1/d (+9 us), GpSimd partition_broadcast (wrong results at
partition offset 64), DVE reciprocal in any shape (+10 us/band), and
aligned-partition gather at rows 0/32/64/96 with 2 wide reciprocals
(+30 us), AV pair-merged PSUM (+2 us), late output projection
(+7 us). no_tail ablation floor is ~268 us; full kernel ~345-360 us.
"""

import contextlib
import sys

if '/opt/trn_rl_repo' not in sys.path:
    sys.path.insert(0, '/opt/trn_rl_repo')

import numpy as np

P = 128
MM_DTYPE = "f32r"
NSEQ = 512
D = 512
H = 8
DH = 64
NBANDS = 7
NCORES = 8
SCALE = D ** -0.5

_cached = None


def _emit_qkv_group(ctx, qk_sb, xt, et):
    """One q/k projection group: [128, n] output tile for head-half et."""
    nc, f32 = ctx["nc"], ctx["f32"]
    pl = ctx["pools"]
    wq_sb = ctx["wq_sb"]
    ps = pl["psproj"].tile([P, NSEQ], f32, tag="psproj")
    for kt in range(4):
        nc.tensor.matmul(
            ps[:], wq_sb[:, kt, et * P:(et + 1) * P], xt[:, kt, :],
            start=(kt == 0), stop=(kt == 3))
    nc.vector.tensor_copy(qk_sb[:, et, :], ps[:])


def _emit_v_group(ctx, v_aug, xt, nt):
    """One v projection group: 128 tokens of row-major v_aug."""
    nc, f32 = ctx["nc"], ctx["f32"]
    pl = ctx["pools"]
    wq_sb = ctx["wq_sb"]
    ps = pl["psproj"].tile([P, NSEQ], f32, tag="psproj")
    for kt in range(4):
        nc.tensor.matmul(
            ps[:], xt[:, kt, nt * P:(nt + 1) * P],
            wq_sb[:, kt, 2 * D:3 * D],
            start=(kt == 0), stop=(kt == 3))
    nc.vector.tensor_copy(
        v_aug[:, nt, :, 0:DH],
        ps[:].rearrange("p (h dh) -> p h dh", h=H))
    ones_slice = v_aug[:, nt, :, DH:DH + 1]
    if ctx["mm_dtype"] == "f32r":
        ones_slice = ones_slice.bitcast(f32)
    nc.vector.memset(ones_slice, 1.0)


def _emit_pair_s(ctx, st, g):
    """S + exp for head pair (2g, 2g+1): returns es tiles."""
    nc, f32, f32r, Exp = ctx["nc"], ctx["f32"], ctx["f32r"], ctx["Exp"]
    pl = ctx["pools"]
    qk_sb = st["qk_sb"]
    es_list = []
    for jt in range(4):
        if ctx["merge_exp"]:
            ps_s = pl["pss"].tile([P, 2, NSEQ], f32, tag="pss")
            ps_a, ps_b = ps_s[:, 0, :], ps_s[:, 1, :]
        else:
            ps_s0 = pl["pss"].tile([P, NSEQ], f32, tag="pss")
            ps_s1 = pl["pss"].tile([P, NSEQ], f32, tag="pss")
            ps_a, ps_b = ps_s0[:], ps_s1[:]
        nc.tensor.matmul(
            ps_a,
            qk_sb[0:DH, 4 + g, jt * P:(jt + 1) * P],
            qk_sb[0:DH, g, :], start=True, stop=True)
        nc.tensor.matmul(
            ps_b,
            qk_sb[DH:P, 4 + g, jt * P:(jt + 1) * P],
            qk_sb[DH:P, g, :], start=True, stop=True,
            tile_position=(DH, 0))
        es = pl["es"].tile([P, 2, NSEQ], f32r, tag="es")
        if ctx["merge_exp"]:
            nc.scalar.activation(es[:], ps_s[:], Exp, scale=SCALE)
        else:
            nc.scalar.activation(es[:, 0, :], ps_a, Exp, scale=SCALE)
            nc.scalar.activation(es[:, 1, :], ps_b, Exp, scale=SCALE)
        es_list.append(es)
    return es_list


def _emit_pair_av(ctx, st, g, es_list):
    """AV + PSUM drain + 1/d for head pair (2g, 2g+1)."""
    nc, f32 = ctx["nc"], ctx["f32"]
    pl = ctx["pools"]
    v_aug = st["v_aug"]
    if ctx["ablate"] == "no_av":
        nc.vector.tensor_copy(st["o_sb"][:, g, :], es_list[0][:, 0, :])
        return
    if ctx["av_merge"]:
        ps_o = pl["pso"].tile([DH + 1, 2, NSEQ], f32, tag="pso")
        ps_o0, ps_o1 = ps_o[:, 0, :], ps_o[:, 1, :]
    else:
        ps_t0 = pl["pso"].tile([DH + 1, NSEQ], f32, tag="pso")
        ps_t1 = pl["pso"].tile([DH + 1, NSEQ], f32, tag="pso")
        ps_o0, ps_o1 = ps_t0[:], ps_t1[:]
    for jt in range(4):
        nc.tensor.matmul(
            ps_o0, v_aug[:, jt, 2 * g, :], es_list[jt][:, 0, :],
            start=(jt == 0), stop=(jt == 3))
        nc.tensor.matmul(
            ps_o1, v_aug[:, jt, 2 * g + 1, :], es_list[jt][:, 1, :],
            start=(jt == 0), stop=(jt == 3))
    o_sb = st["o_sb"]
    nc.vector.tensor_copy(o_sb[0:DH, g, :], ps_o0[0:DH, :])
    nc.vector.tensor_copy(o_sb[DH:P, g, :], ps_o1[0:DH, :])
    if ctx["ablate"] != "no_tail":
        # parity-major layout so the broadcast is a single DMA later
        if ctx["tail"] == "gather":
            # denominator rows to aligned partitions 32g of two [97, n]
            # tiles: one reciprocal per parity covers 4 heads (DVE op
            # cost is free-size-bound, partition count is free)
            nc.vector.tensor_copy(st["dp0"][32 * g:32 * g + 1, :],
                                  ps_o0[DH:DH + 1, :])
            nc.vector.tensor_copy(st["dp1"][32 * g:32 * g + 1, :],
                                  ps_o1[DH:DH + 1, :])
        elif ctx["recip"] == "bcast_act":
            # gather RAW denominators on DVE (off ACT's exp stream);
            # 1/d is computed partition-parallel on the broadcast tile
            rcc = st["rcc"]
            if ctx["av_merge"]:
                nc.vector.tensor_copy(rcc[0:1, :, g, :],
                                      ps_o[DH:DH + 1, :, :])
            else:
                nc.vector.tensor_copy(rcc[0:1, 0, g, :],
                                      ps_o0[DH:DH + 1, :])
                nc.vector.tensor_copy(rcc[0:1, 1, g, :],
                                      ps_o1[DH:DH + 1, :])
            if "dr" in st:
                # last band: bounce each pair's denominators as soon as
                # they exist so the loop-epilogue drain skips the DMA leg
                dr, rb = st["dr"], st["rb"]
                nc.sync.dma_start(dr[0:1, :, g, :], rcc[0:1, :, g, :])
                nc.sync.dma_start(
                    rb[0:DH, g, :],
                    dr[0:1, 0, g, :].to_broadcast((DH, NSEQ)))
                nc.sync.dma_start(
                    rb[DH:P, g, :],
                    dr[0:1, 1, g, :].to_broadcast((DH, NSEQ)))
        elif ctx["recip"] == "dve":
            rcc = st["rcc"]
            nc.vector.reciprocal(rcc[0:1, 0, g, :], ps_o0[DH:DH + 1, :])
            nc.vector.reciprocal(rcc[0:1, 1, g, :], ps_o1[DH:DH + 1, :])
        else:
            # ln into the band tile; one batched exp(-ln d) at tail time
            # (ln+exp share one ACT table set -> no table reloads)
            lg = st["lg"]
            nc.scalar.activation(lg[0:1, 0, g, :], ps_o0[DH:DH + 1, :],
                                 ctx["Ln"])
            nc.scalar.activation(lg[0:1, 1, g, :], ps_o1[DH:DH + 1, :],
                                 ctx["Ln"])


def _emit_tail_dma(ctx, pend):
    """Start the lagged broadcast of 1/d for band pend['s']: one DRAM
    bounce + one stride-0 broadcast DMA into [128, 4, n]."""
    nc, f32 = ctx["nc"], ctx["f32"]
    pl = ctx["pools"]
    if ctx["ablate"] in ("no_tail", "no_attn", "tail_act"):
        return
    if ctx["recip"] == "bcast_act":
        if "rb" in pend:
            rb = pend["rb"]  # last band: broadcast already in flight
        else:
            dr = pl["dram"].tile([1, 2, 4, NSEQ], f32, tag="dr")
            nc.sync.dma_start(dr[:], pend["rcc"][:])
            rb = pl["rb"].tile([P, 4, NSEQ], f32, tag="rb")
            for g in range(4):
                nc.sync.dma_start(
                    rb[0:DH, g, :],
                    dr[0:1, 0, g, :].to_broadcast((DH, NSEQ)))
                nc.sync.dma_start(
                    rb[DH:P, g, :],
                    dr[0:1, 1, g, :].to_broadcast((DH, NSEQ)))
        # 1/d = exp(-ln d), partition-parallel over the broadcast: two
        # [128, 2048] ACT ops replace 8 single-lane Lns + a single-lane
        # [1, 4096] exp batch (ln+exp share one table set)
        rbl = pl["rbl"].tile([P, 4, NSEQ], f32, tag="rbl")
        nc.scalar.activation(rbl[:], rb[:], ctx["Ln"])
        nc.scalar.activation(rb[:], rbl[:], ctx["Exp"], scale=-1.0)
        pend["rb"] = rb
        return
    if ctx["tail"] == "gather":
        rc0 = pl["rc2"].tile([97, NSEQ], f32, tag="rc0")
        rc1 = pl["rc2"].tile([97, NSEQ], f32, tag="rc1")
        nc.vector.reciprocal(rc0[:], pend["dp0"][:])
        nc.vector.reciprocal(rc1[:], pend["dp1"][:])
        dr0 = pl["dram"].tile([97, NSEQ], f32, tag="dr0")
        dr1 = pl["dram"].tile([97, NSEQ], f32, tag="dr1")
        nc.sync.dma_start(dr0[:], rc0[:])
        nc.sync.dma_start(dr1[:], rc1[:])
        rb = pl["rb"].tile([P, 4, NSEQ], f32, tag="rb")
        for g in range(4):
            nc.sync.dma_start(
                rb[0:DH, g, :],
                dr0[32 * g:32 * g + 1, :].to_broadcast((DH, NSEQ)))
            nc.sync.dma_start(
                rb[DH:P, g, :],
                dr1[32 * g:32 * g + 1, :].to_broadcast((DH, NSEQ)))
        pend["rb"] = rb
        return
    if ctx["recip"] != "dve":
        nc.scalar.activation(pend["rcc"][:], pend["lg"][:], ctx["Exp"],
                             scale=-1.0)
    if ctx["tail"] == "pemask":
        return
    dr = pl["dram"].tile([1, 2, 4, NSEQ], f32, tag="dr")
    nc.sync.dma_start(dr[:], pend["rcc"][:])
    rb = pl["rb"].tile([P, 4, NSEQ], f32, tag="rb")
    if ctx["tail"] == "dma_split":
        # several smaller DMAs spread across queues/engines
        for g in range(4):
            nc.sync.dma_start(
                rb[0:DH, g, :],
                dr[0:1, 0, g, :].to_broadcast((DH, NSEQ)))
            nc.sync.dma_start(
                rb[DH:P, g, :],
                dr[0:1, 1, g, :].to_broadcast((DH, NSEQ)))
    else:
        nc.sync.dma_start(
            rb[:], dr[0].unsqueeze(1).broadcast_to((2, DH, 4, NSEQ)))
    pend["rb"] = rb


def _emit_tail_muls(ctx, pend):
    """Normalize band pend['s']'s O^T by the broadcast 1/d."""
    nc = ctx["nc"]
    pl = ctx["pools"]
    if ctx["ablate"] in ("no_tail", "no_attn", "tail_act", "tail_dma"):
        return
    if ctx["mul_inplace"]:
        o_sb = pend["o_sb"]
        nc.vector.tensor_mul(o_sb[:], o_sb[:], pend["rb"][:])
        return
    if ctx["mul_gpsimd"]:
        # idle engine, fires as soon as the broadcast lands — never
        # queues behind the projection-phase PSUM drains on DVE
        ot = pl["ot"].tile([P, 4, NSEQ], ctx["f32"], tag="ot")
        nc.gpsimd.tensor_mul(ot[:], pend["o_sb"][:].bitcast(ctx["f32"]),
                             pend["rb"][:])
        pend["o_sb"] = ot[:].bitcast(ctx["f32r"])
        return
    ot = pl["ot"].tile([P, 4, NSEQ], ctx["f32r"], tag="ot")
    if ctx["tail"] == "pemask":
        # materialize 1/d across partitions with two K=1 mask matmuls
        # per pair-group (PE, ~213ns each; no DMA round trip at all),
        # multiply straight out of PSUM
        rcc, o_sb = pend["rcc"], pend["o_sb"]
        for g in range(4):
            rb_ps = pl["pso"].tile([P, NSEQ], ctx["f32"], tag="pso")
            nc.tensor.matmul(rb_ps[:], ctx["maskA"][:],
                             rcc[0:1, 0, g, :], start=True, stop=False)
            nc.tensor.matmul(rb_ps[:], ctx["maskB"][:],
                             rcc[0:1, 1, g, :], start=False, stop=True)
            nc.vector.tensor_mul(ot[:, g, :], o_sb[:, g, :], rb_ps[:])
    else:
        nc.vector.tensor_mul(ot[:], pend["o_sb"][:], pend["rb"][:])
    pend["o_sb"] = ot


def _emit_outproj_group(ctx, pend, nt):
    """One output-projection group (128 tokens) + bias for one band."""
    nc, f32 = ctx["nc"], ctx["f32"]
    wo_sb, bias_sb = ctx["wo_sb"], ctx["bias_sb"]
    pl = ctx["pools"]
    o_sb = pend["o_sb"]
    if "ob" not in pend:
        ob = pl["ob"].tile([P, 4, D], f32, tag="ob")
        pend["ob"] = ob
    ps = pl["psproj"].tile([P, NSEQ], f32, tag="psproj")
    for kt in range(4):
        nc.tensor.matmul(
            ps[:], o_sb[:, kt, nt * P:(nt + 1) * P], wo_sb[:, kt, :],
            start=(kt == 0), stop=(kt == 3))
    nc.vector.tensor_add(pend["ob"][:, nt, :], ps[:], bias_sb[:])


def _emit_outproj_store(ctx, pend):
    nc = ctx["nc"]
    dst = ctx["out"][pend["s"]].rearrange("(no ni) e -> ni no e", ni=P)
    if ctx["tail"] == "dma_split":
        for nt in range(4):
            nc.sync.dma_start(dst[:, nt, :], pend["ob"][:, nt, :])
    else:
        nc.sync.dma_start(dst, pend["ob"][:])


def _emit_outproj(ctx, pend):
    """Output projection + bias + single batched store for one band."""
    for nt in range(4):
        _emit_outproj_group(ctx, pend, nt)
    _emit_outproj_store(ctx, pend)


def build_kernel(nbands=NBANDS, repeat=1, mm_dtype=MM_DTYPE, ablate="",
                 psum=(2, 2, 2), es_bufs=8, recip="bcast_act", merge_exp=True,
                 tail="dma_split", x_split=False, late_outproj=False,
                 av_merge=False, mul_inplace=False, v_in_attn=False,
                 early_last=False, mul_gpsimd=False):
    import concourse.mybir as mybir
    import concourse.tile as tile
    from concourse import bacc
    from concourse import library_config

    f32 = mybir.dt.float32
    f32r = (mybir.dt.float32r if mm_dtype == "f32r" else mybir.dt.bfloat16)
    Exp = mybir.ActivationFunctionType.Exp
    Ln = mybir.ActivationFunctionType.Ln

    nc = bacc.Bacc("TRN2", target_bir_lowering=False, debug=False,
                   num_devices=NCORES)

    xT = nc.dram_tensor("xT", [nbands, D, NSEQ], f32r, kind="ExternalInput").ap()
    wqkvT = nc.dram_tensor("wqkvT", [D, 3 * D], f32r, kind="ExternalInput").ap()
    woutT = nc.dram_tensor("woutT", [D, D], f32r, kind="ExternalInput").ap()
    biasb = nc.dram_tensor("biasb", [P, D], f32, kind="ExternalInput").ap()
    out = nc.dram_tensor("out", [nbands, NSEQ, D], f32, kind="ExternalOutput").ap()

    nc.gpsimd.load_library(library_config.attn)

    with tile.TileContext(nc) as tc:
        with contextlib.ExitStack() as _stack:
            _p = lambda *a, **kw: _stack.enter_context(tc.tile_pool(*a, **kw))
            wpool = _p(name="weights", bufs=1)
            xpool = _p(name="x", bufs=2)
            qkpool = _p(name="qk", bufs=2)
            vpool = _p(name="v", bufs=2)
            osbpool = _p(name="osb", bufs=2)
            spool = _p(name="es", bufs=es_bufs)
            rccpool = _p(name="rcc", bufs=(2 if recip == "dve" else 1))
            dppool = _p(name="dp", bufs=2)
            rblpool = _p(name="rbl", bufs=1)
            rc2pool = _p(name="rc2", bufs=1)
            lgpool = _p(name="lg", bufs=1)
            rbpool = _p(name="rb", bufs=1)
            otpool = _p(name="ot", bufs=1)
            drampool = _p(name="dram", bufs=2, space="DRAM")
            outpool = _p(name="ob", bufs=1)
            psproj = _p(name="psproj", bufs=psum[0], space="PSUM")
            pss = _p(name="pss", bufs=psum[1], space="PSUM")
            pso = _p(name="pso", bufs=psum[2], space="PSUM")
            # weights: split wq by k-chunk so the first matmuls can start
            # as soon as their chunk lands
            wq_sb = wpool.tile([P, 4, 3 * D], f32r)
            wo_sb = wpool.tile([P, 4, D], f32r)
            bias_sb = wpool.tile([P, D], f32)
            maskA = wpool.tile([1, P], f32)
            maskB = wpool.tile([1, P], f32)
            nc.vector.memset(maskA[:], 0.0)
            nc.vector.memset(maskB[:], 0.0)
            nc.vector.memset(maskA[0:1, 0:DH], 1.0)
            nc.vector.memset(maskB[0:1, DH:P], 1.0)
            wq_r = wqkvT.rearrange("(ko ki) e -> ki ko e", ki=P)
            for kt in range(4):
                nc.sync.dma_start(wq_sb[:, kt, :], wq_r[:, kt, :])
            nc.sync.dma_start(wo_sb[:], woutT.rearrange("(ko ki) e -> ki ko e", ki=P))
            nc.sync.dma_start(bias_sb[:], biasb[:])

            ctx = {
                "nc": nc, "f32": f32, "f32r": f32r, "Exp": Exp, "Ln": Ln,
                "mm_dtype": mm_dtype, "ablate": ablate, "recip": recip,
                "merge_exp": merge_exp, "tail": tail, "av_merge": av_merge,
                "mul_inplace": mul_inplace, "mul_gpsimd": mul_gpsimd,
                "wq_sb": wq_sb, "wo_sb": wo_sb, "bias_sb": bias_sb,
                "maskA": maskA, "maskB": maskB,
                "out": out,
                "pools": {
                    "qk": qkpool, "v": vpool, "osb": osbpool, "es": spool,
                    "rcc": rccpool, "rb": rbpool, "rc2": rc2pool,
                    "rbl": rblpool,
                    "ot": otpool, "dram": drampool,
                    "ob": outpool, "psproj": psproj,
                    "pss": pss, "pso": pso,
                },
            }

            def load_x(s):
                xt = xpool.tile([P, 4, NSEQ], f32r, tag="xt")
                xv = xT[s].rearrange("(ko ki) n -> ki ko n", ki=P)
                if x_split:
                    for kt in range(4):
                        nc.sync.dma_start(xt[:, kt, :], xv[:, kt, :])
                else:
                    nc.sync.dma_start(xt[:], xv)
                return xt

            def new_proj_tiles():
                qk_sb = qkpool.tile([P, 8, NSEQ], f32r, tag="qk")
                v_aug = vpool.tile([P, 4, H, DH + 1], f32r, tag="vaug")
                return qk_sb, v_aug

            def proj_thunks(qk_sb, v_aug, xt):
                th = [(lambda e=et: _emit_qkv_group(ctx, qk_sb, xt, e))
                      for et in (0, 4, 1, 5, 2, 6, 3, 7)]
                th += [(lambda n=nt: _emit_v_group(ctx, v_aug, xt, n))
                       for nt in range(4)]
                return th

            def new_attn_state(s, qk_sb, v_aug):
                o_sb = osbpool.tile([P, 4, NSEQ], f32r, tag="osb")
                st = {"s": s, "qk_sb": qk_sb, "v_aug": v_aug, "o_sb": o_sb}
                if recip == "bcast_act":
                    rcc = rccpool.tile([1, 2, 4, NSEQ], f32, tag="rcc")
                    st["rcc"] = rcc
                elif tail == "gather":
                    dp0 = dppool.tile([97, NSEQ], f32, tag="dp0")
                    dp1 = dppool.tile([97, NSEQ], f32, tag="dp1")
                    nc.vector.memset(dp0[:], 1.0)
                    nc.vector.memset(dp1[:], 1.0)
                    st["dp0"], st["dp1"] = dp0, dp1
                else:
                    rcc = rccpool.tile([1, 2, 4, NSEQ], f32, tag="rcc")
                    lg = lgpool.tile([1, 2, 4, NSEQ], f32, tag="lg")
                    st["rcc"], st["lg"] = rcc, lg
                return st

            rep_ctx = (tc.For_i(0, repeat, 1,
                                hint_engines=(mybir.EngineType.PE,
                                              mybir.EngineType.Activation,
                                              mybir.EngineType.DVE))
                       if repeat > 1 else contextlib.nullcontext())
            with rep_ctx:
                # prefetch x one band ahead; tail+outproj lag one band.
                # Schedule per band s: [tail-bounce DMAs for s-1 issued
                # first] qk groups, [normalize mul for s-1 on DVE where
                # its broadcast has landed], v groups, [outproj s-1],
                # attention pairs. Fine-grained interleaving of proj
                # groups into the attention pairs measured SLOWER on HW
                # (cross-engine interlock overhead beats exp-wait fill).
                xt_next = load_x(0)
                pend = None
                for s in range(nbands):
                    xt = xt_next
                    if s + 1 < nbands:
                        xt_next = load_x(s + 1)
                    if pend is not None:
                        _emit_tail_dma(ctx, pend)
                    qk_sb, v_aug = new_proj_tiles()
                    groups = proj_thunks(qk_sb, v_aug, xt)
                    for th in groups[0:8]:
                        th()
                    if pend is not None:
                        _emit_tail_muls(ctx, pend)
                    if not v_in_attn:
                        for th in groups[8:12]:
                            th()
                    if pend is not None and not late_outproj:
                        _emit_outproj(ctx, pend)
                    st = new_attn_state(s, qk_sb, v_aug)
                    if (early_last and s == nbands - 1
                            and recip == "bcast_act" and ablate == ""):
                        dr6 = drampool.tile([1, 2, 4, NSEQ], f32, tag="dr")
                        rb6 = rbpool.tile([P, 4, NSEQ], f32, tag="rb")
                        st["dr"], st["rb"] = dr6, rb6
                    if ctx["ablate"] == "no_attn":
                        nc.vector.tensor_copy(st["o_sb"][:],
                                              st["qk_sb"][:, 0:4, :])
                        if v_in_attn:
                            for th in groups[8:12]:
                                th()
                        if pend is not None and late_outproj:
                            _emit_outproj(ctx, pend)
                    else:
                        for g in range(4):
                            es_list = _emit_pair_s(ctx, st, g)
                            if v_in_attn and g == 0:
                                for th in groups[8:12]:
                                    th()
                            _emit_pair_av(ctx, st, g, es_list)
                            if g == 0 and pend is not None and late_outproj:
                                _emit_outproj(ctx, pend)
                    pend = st
                _emit_tail_dma(ctx, pend)
                _emit_tail_muls(ctx, pend)
                _emit_outproj(ctx, pend)

    nc.compile()
    return nc


def _get_nc():
    global _cached
    if _cached is None:
        _cached = build_kernel()
    return _cached


def make_in_maps(x, x_delta, x_theta, x_alpha, x_beta, x_gamma, x_upper,
                 Wqkv, Wout, bout, mm_dtype=MM_DTYPE):
    if mm_dtype == "f32r":
        cast_dt = np.float32
    else:
        import ml_dtypes
        cast_dt = ml_dtypes.bfloat16
    xs = np.stack([np.asarray(a, dtype=np.float32) for a in
                   (x, x_delta, x_theta, x_alpha, x_beta, x_gamma, x_upper)],
                  axis=0)  # [7, b, n, d]
    xsT = np.ascontiguousarray(xs.transpose(1, 0, 3, 2).astype(cast_dt))
    wqkvT = np.ascontiguousarray(np.asarray(Wqkv, np.float32).T.astype(cast_dt))
    woutT = np.ascontiguousarray(np.asarray(Wout, np.float32).T.astype(cast_dt))
    biasb = np.ascontiguousarray(
        np.broadcast_to(np.asarray(bout, np.float32)[None, :], (P, D)))
    return [
        {"xT": xsT[c], "wqkvT": wqkvT, "woutT": woutT, "biasb": biasb}
        for c in range(NCORES)
    ]


def kernel(x, x_delta, x_theta, x_alpha, x_beta, x_gamma, x_upper,
           Wqkv, Wout, bout):
    from concourse.bass_utils import run_bass_kernel_spmd

    nc = _get_nc()
    in_maps = make_in_maps(x, x_delta, x_theta, x_alpha, x_beta, x_gamma,
                           x_upper, Wqkv, Wout, bout)
    res = run_bass_kernel_spmd(nc, in_maps, core_ids=list(range(NCORES)))
    full = np.empty((NBANDS, NCORES, NSEQ, D), dtype=np.float32)
    for c in range(NCORES):
        full[:, c] = res.results[c]["out"]
    return tuple(full[i] for i in range(NBANDS))


# revision 67
# speedup vs baseline: 1.1447x; 1.1447x over previous
"""Trainium2 Bass kernel for nn_Attention_86268713108190.

7 independent attention "bands" over batch 8, n=512, d=512, 8 heads,
shared Wqkv/Wout. Sharding: data-parallel over batch — core c handles
batch index c (7 band-samples of [512, 512] each).

Per-core dataflow (per band; all matmuls in float32r, which streams
1 row/cycle at ~2.4 GHz for moving size >= 256, same as bf16; PE
matmul time = out-free-size rows, independent of K and M):
  1. qT/kT = Wqkv @ x^T  (lhsT = WqkvT chunks, rhs = x^T)   [e, n]
  2. v    = x @ Wv^T     (lhsT = x^T chunks,   rhs = WvT)   [n, ev]
     v_aug: per head 64 v-cols + a ones column (65) -> softmax
     denominator falls out of the AV matmul for free
  3. per head pair: S^T = k_h q_h^T (two K=64 matmuls packed into one
     2-bank PSUM tile via tile_position), ONE expS^T = exp(SCALE*S^T)
     activation per jt ([128, 1024] — halving ACT op count measurably
     beats separate [128, 512] exps; no max-subtraction needed:
     |SCALE*S| <~ 1.1 for this distribution), O_aug^T[65, n] =
     v_aug.T @ expS^T accumulated over j-tiles; row 64 = softmax
     denominator. Per pair, DVE copies the RAW denominator rows into a
     parity-major [1, 2, 4, n] tile and the unnormalized O^T to SBUF
     (frees PSUM fast; keeps ACT's in-order exp stream untouched —
     putting Ln/reciprocal ops there measured +35-45 us).
  4. batched tail, lagged one full band: one DRAM bounce, then 8
     stride-0 broadcast DMAs materialize raw d across [128, 4, n]
     (kept as 8 SEPARATE DMAs on purpose — one big batched DMA
     serializes on a single DMA engine/queue, ~25 us slower; SBUF
     stride-0 APs are illegal and GpSimd partition_broadcast cannot
     write at a nonzero partition offset on HW; DRAM-source stride-0
     DMAs lower fine and DMA writes any partition range). 1/d =
     exp(-ln d) is then computed PARTITION-PARALLEL on the broadcast
     tile: two [128, 2048] ACT ops during the (ACT-idle) projection
     phase replace 8 single-lane Lns + a [1, 4096] single-lane exp
     in the exp stream (-45 us; DVE reciprocal is ~1.5-2 us/op on HW,
     far above its cost model — unusable in any shape). One
     [128, 4, n] DVE multiply normalizes O^T.
  5. out = O @ Wout^T + bias for the lagged band, batched store.

Schedule per band s: [bounce DMAs + broadcast exp(-ln) for band s-1
issued first] 8 qk groups, [normalize mul for s-1 on DVE, broadcast
landed by now], 4 v groups, [output projection of s-1], 4 attention
pairs.
Measured dead ends on this HW: fine-grained interleaving of band s+1
projection groups into band s's attention pairs (cross-engine
interlock overhead beats the exp-wait fill: +40-75 us), PE mask-matmul
broadcast of 1/d (+9 us), GpSimd partition_broadcast (wrong results at
partition offset 64), DVE reciprocal in any shape (+10 us/band), and
aligned-partition gather at rows 0/32/64/96 with 2 wide reciprocals
(+30 us), AV pair-merged PSUM (+2 us), late output projection
(+7 us). no_tail ablation floor is ~268 us; full kernel ~345-360 us.
"""

import contextlib
import sys

if '/opt/trn_rl_repo' not in sys.path:
    sys.path.insert(0, '/opt/trn_rl_repo')

import numpy as np

P = 128
MM_DTYPE = "f32r"
NSEQ = 512
D = 512
H = 8
DH = 64
NBANDS = 7
NCORES = 8
SCALE = D ** -0.5

_cached = None


def _emit_qkv_group(ctx, qk_sb, xt, et):
    """One q/k projection group: [128, n] output tile for head-half et."""
    nc, f32 = ctx["nc"], ctx["f32"]
    pl = ctx["pools"]
    wq_sb = ctx["wq_sb"]
    ps = pl["psproj"].tile([P, NSEQ], f32, tag="psproj")
    for kt in range(4):
        nc.tensor.matmul(
            ps[:], wq_sb[:, kt, et * P:(et + 1) * P], xt[:, kt, :],
            start=(kt == 0), stop=(kt == 3))
    nc.vector.tensor_copy(qk_sb[:, et, :], ps[:])


def _emit_v_group(ctx, v_aug, xt, nt):
    """One v projection group: 128 tokens of row-major v_aug."""
    nc, f32 = ctx["nc"], ctx["f32"]
    pl = ctx["pools"]
    wq_sb = ctx["wq_sb"]
    ps = pl["psproj"].tile([P, NSEQ], f32, tag="psproj")
    for kt in range(4):
        nc.tensor.matmul(
            ps[:], xt[:, kt, nt * P:(nt + 1) * P],
            wq_sb[:, kt, 2 * D:3 * D],
            start=(kt == 0), stop=(kt == 3))
    nc.vector.tensor_copy(
        v_aug[:, nt, :, 0:DH],
        ps[:].rearrange("p (h dh) -> p h dh", h=H))
    ones_slice = v_aug[:, nt, :, DH:DH + 1]
    if ctx["mm_dtype"] == "f32r":
        ones_slice = ones_slice.bitcast(f32)
    nc.vector.memset(ones_slice, 1.0)


def _emit_pair_s(ctx, st, g):
    """S + exp for head pair (2g, 2g+1): returns es tiles."""
    nc, f32, f32r, Exp = ctx["nc"], ctx["f32"], ctx["f32r"], ctx["Exp"]
    pl = ctx["pools"]
    qk_sb = st["qk_sb"]
    es_list = []
    for jt in range(4):
        if ctx["merge_exp"]:
            ps_s = pl["pss"].tile([P, 2, NSEQ], f32, tag="pss")
            ps_a, ps_b = ps_s[:, 0, :], ps_s[:, 1, :]
        else:
            ps_s0 = pl["pss"].tile([P, NSEQ], f32, tag="pss")
            ps_s1 = pl["pss"].tile([P, NSEQ], f32, tag="pss")
            ps_a, ps_b = ps_s0[:], ps_s1[:]
        nc.tensor.matmul(
            ps_a,
            qk_sb[0:DH, 4 + g, jt * P:(jt + 1) * P],
            qk_sb[0:DH, g, :], start=True, stop=True)
        nc.tensor.matmul(
            ps_b,
            qk_sb[DH:P, 4 + g, jt * P:(jt + 1) * P],
            qk_sb[DH:P, g, :], start=True, stop=True,
            tile_position=(DH, 0))
        es = pl["es"].tile([P, 2, NSEQ], f32r, tag="es")
        if ctx["merge_exp"]:
            nc.scalar.activation(es[:], ps_s[:], Exp, scale=SCALE)
        else:
            nc.scalar.activation(es[:, 0, :], ps_a, Exp, scale=SCALE)
            nc.scalar.activation(es[:, 1, :], ps_b, Exp, scale=SCALE)
        es_list.append(es)
    return es_list


def _emit_pair_av(ctx, st, g, es_list):
    """AV + PSUM drain + 1/d for head pair (2g, 2g+1)."""
    nc, f32 = ctx["nc"], ctx["f32"]
    pl = ctx["pools"]
    v_aug = st["v_aug"]
    if ctx["ablate"] == "no_av":
        nc.vector.tensor_copy(st["o_sb"][:, g, :], es_list[0][:, 0, :])
        return
    if ctx["av_merge"]:
        ps_o = pl["pso"].tile([DH + 1, 2, NSEQ], f32, tag="pso")
        ps_o0, ps_o1 = ps_o[:, 0, :], ps_o[:, 1, :]
    else:
        ps_t0 = pl["pso"].tile([DH + 1, NSEQ], f32, tag="pso")
        ps_t1 = pl["pso"].tile([DH + 1, NSEQ], f32, tag="pso")
        ps_o0, ps_o1 = ps_t0[:], ps_t1[:]
    for jt in range(4):
        nc.tensor.matmul(
            ps_o0, v_aug[:, jt, 2 * g, :], es_list[jt][:, 0, :],
            start=(jt == 0), stop=(jt == 3))
        nc.tensor.matmul(
            ps_o1, v_aug[:, jt, 2 * g + 1, :], es_list[jt][:, 1, :],
            start=(jt == 0), stop=(jt == 3))
    o_sb = st["o_sb"]
    nc.vector.tensor_copy(o_sb[0:DH, g, :], ps_o0[0:DH, :])
    nc.vector.tensor_copy(o_sb[DH:P, g, :], ps_o1[0:DH, :])
    if ctx["ablate"] != "no_tail":
        # parity-major layout so the broadcast is a single DMA later
        if ctx["tail"] == "gather":
            # denominator rows to aligned partitions 32g of two [97, n]
            # tiles: one reciprocal per parity covers 4 heads (DVE op
            # cost is free-size-bound, partition count is free)
            nc.vector.tensor_copy(st["dp0"][32 * g:32 * g + 1, :],
                                  ps_o0[DH:DH + 1, :])
            nc.vector.tensor_copy(st["dp1"][32 * g:32 * g + 1, :],
                                  ps_o1[DH:DH + 1, :])
        elif ctx["recip"] == "bcast_act":
            # gather RAW denominators on DVE (off ACT's exp stream);
            # 1/d is computed partition-parallel on the broadcast tile
            rcc = st["rcc"]
            if ctx["av_merge"]:
                nc.vector.tensor_copy(rcc[0:1, :, g, :],
                                      ps_o[DH:DH + 1, :, :])
            else:
                nc.vector.tensor_copy(rcc[0:1, 0, g, :],
                                      ps_o0[DH:DH + 1, :])
                nc.vector.tensor_copy(rcc[0:1, 1, g, :],
                                      ps_o1[DH:DH + 1, :])
            if "dr" in st:
                # last band: bounce each pair's denominators as soon as
                # they exist so the loop-epilogue drain skips the DMA leg
                dr, rb = st["dr"], st["rb"]
                nc.sync.dma_start(dr[0:1, :, g, :], rcc[0:1, :, g, :])
                nc.sync.dma_start(
                    rb[0:DH, g, :],
                    dr[0:1, 0, g, :].to_broadcast((DH, NSEQ)))
                nc.sync.dma_start(
                    rb[DH:P, g, :],
                    dr[0:1, 1, g, :].to_broadcast((DH, NSEQ)))
        elif ctx["recip"] == "dve":
            rcc = st["rcc"]
            nc.vector.reciprocal(rcc[0:1, 0, g, :], ps_o0[DH:DH + 1, :])
            nc.vector.reciprocal(rcc[0:1, 1, g, :], ps_o1[DH:DH + 1, :])
        else:
            # ln into the band tile; one batched exp(-ln d) at tail time
            # (ln+exp share one ACT table set -> no table reloads)
            lg = st["lg"]
            nc.scalar.activation(lg[0:1, 0, g, :], ps_o0[DH:DH + 1, :],
                                 ctx["Ln"])
            nc.scalar.activation(lg[0:1, 1, g, :], ps_o1[DH:DH + 1, :],
                                 ctx["Ln"])


def _emit_tail_dma(ctx, pend):
    """Start the lagged broadcast of 1/d for band pend['s']: one DRAM
    bounce + one stride-0 broadcast DMA into [128, 4, n]."""
    nc, f32 = ctx["nc"], ctx["f32"]
    pl = ctx["pools"]
    if ctx["ablate"] in ("no_tail", "no_attn", "tail_act"):
        return
    if ctx["recip"] == "bcast_act":
        if "rb" in pend:
            rb = pend["rb"]  # last band: broadcast already in flight
        else:
            dr = pl["dram"].tile([1, 2, 4, NSEQ], f32, tag="dr")
            nc.sync.dma_start(dr[:], pend["rcc"][:])
            rb = pl["rb"].tile([P, 4, NSEQ], f32, tag="rb")
            for g in range(4):
                # split issue across SP and the idle GpSimd SWDGE to
                # halve descriptor-issue serialization on this chain
                eng = (nc.gpsimd if (ctx["dma_mixed"] and g >= 2)
                       else nc.sync)
                eng.dma_start(
                    rb[0:DH, g, :],
                    dr[0:1, 0, g, :].to_broadcast((DH, NSEQ)))
                eng.dma_start(
                    rb[DH:P, g, :],
                    dr[0:1, 1, g, :].to_broadcast((DH, NSEQ)))
        # 1/d = exp(-ln d), partition-parallel over the broadcast: two
        # [128, 2048] ACT ops replace 8 single-lane Lns + a single-lane
        # [1, 4096] exp batch (ln+exp share one table set)
        rbl = pl["rbl"].tile([P, 4, NSEQ], f32, tag="rbl")
        nc.scalar.activation(rbl[:], rb[:], ctx["Ln"])
        nc.scalar.activation(rb[:], rbl[:], ctx["Exp"], scale=-1.0)
        pend["rb"] = rb
        return
    if ctx["tail"] == "gather":
        rc0 = pl["rc2"].tile([97, NSEQ], f32, tag="rc0")
        rc1 = pl["rc2"].tile([97, NSEQ], f32, tag="rc1")
        nc.vector.reciprocal(rc0[:], pend["dp0"][:])
        nc.vector.reciprocal(rc1[:], pend["dp1"][:])
        dr0 = pl["dram"].tile([97, NSEQ], f32, tag="dr0")
        dr1 = pl["dram"].tile([97, NSEQ], f32, tag="dr1")
        nc.sync.dma_start(dr0[:], rc0[:])
        nc.sync.dma_start(dr1[:], rc1[:])
        rb = pl["rb"].tile([P, 4, NSEQ], f32, tag="rb")
        for g in range(4):
            nc.sync.dma_start(
                rb[0:DH, g, :],
                dr0[32 * g:32 * g + 1, :].to_broadcast((DH, NSEQ)))
            nc.sync.dma_start(
                rb[DH:P, g, :],
                dr1[32 * g:32 * g + 1, :].to_broadcast((DH, NSEQ)))
        pend["rb"] = rb
        return
    if ctx["recip"] != "dve":
        nc.scalar.activation(pend["rcc"][:], pend["lg"][:], ctx["Exp"],
                             scale=-1.0)
    if ctx["tail"] == "pemask":
        return
    dr = pl["dram"].tile([1, 2, 4, NSEQ], f32, tag="dr")
    nc.sync.dma_start(dr[:], pend["rcc"][:])
    rb = pl["rb"].tile([P, 4, NSEQ], f32, tag="rb")
    if ctx["tail"] == "dma_split":
        # several smaller DMAs spread across queues/engines
        for g in range(4):
            nc.sync.dma_start(
                rb[0:DH, g, :],
                dr[0:1, 0, g, :].to_broadcast((DH, NSEQ)))
            nc.sync.dma_start(
                rb[DH:P, g, :],
                dr[0:1, 1, g, :].to_broadcast((DH, NSEQ)))
    else:
        nc.sync.dma_start(
            rb[:], dr[0].unsqueeze(1).broadcast_to((2, DH, 4, NSEQ)))
    pend["rb"] = rb


def _emit_tail_muls(ctx, pend):
    """Normalize band pend['s']'s O^T by the broadcast 1/d."""
    nc = ctx["nc"]
    pl = ctx["pools"]
    if ctx["ablate"] in ("no_tail", "no_attn", "tail_act", "tail_dma"):
        return
    if ctx["mul_inplace"]:
        o_sb = pend["o_sb"]
        nc.vector.tensor_mul(o_sb[:], o_sb[:], pend["rb"][:])
        return
    if ctx["mul_gpsimd"]:
        # idle engine, fires as soon as the broadcast lands — never
        # queues behind the projection-phase PSUM drains on DVE
        ot = pl["ot"].tile([P, 4, NSEQ], ctx["f32"], tag="ot")
        nc.gpsimd.tensor_mul(ot[:], pend["o_sb"][:].bitcast(ctx["f32"]),
                             pend["rb"][:])
        pend["o_sb"] = ot[:].bitcast(ctx["f32r"])
        return
    ot = pl["ot"].tile([P, 4, NSEQ], ctx["f32r"], tag="ot")
    if ctx["tail"] == "pemask":
        # materialize 1/d across partitions with two K=1 mask matmuls
        # per pair-group (PE, ~213ns each; no DMA round trip at all),
        # multiply straight out of PSUM
        rcc, o_sb = pend["rcc"], pend["o_sb"]
        for g in range(4):
            rb_ps = pl["pso"].tile([P, NSEQ], ctx["f32"], tag="pso")
            nc.tensor.matmul(rb_ps[:], ctx["maskA"][:],
                             rcc[0:1, 0, g, :], start=True, stop=False)
            nc.tensor.matmul(rb_ps[:], ctx["maskB"][:],
                             rcc[0:1, 1, g, :], start=False, stop=True)
            nc.vector.tensor_mul(ot[:, g, :], o_sb[:, g, :], rb_ps[:])
    else:
        nc.vector.tensor_mul(ot[:], pend["o_sb"][:], pend["rb"][:])
    pend["o_sb"] = ot


def _emit_outproj_group(ctx, pend, nt):
    """One output-projection group (128 tokens) + bias for one band."""
    nc, f32 = ctx["nc"], ctx["f32"]
    wo_sb, bias_sb = ctx["wo_sb"], ctx["bias_sb"]
    pl = ctx["pools"]
    o_sb = pend["o_sb"]
    if "ob" not in pend:
        ob = pl["ob"].tile([P, 4, D], f32, tag="ob")
        pend["ob"] = ob
    ps = pl["psproj"].tile([P, NSEQ], f32, tag="psproj")
    for kt in range(4):
        nc.tensor.matmul(
            ps[:], o_sb[:, kt, nt * P:(nt + 1) * P], wo_sb[:, kt, :],
            start=(kt == 0), stop=(kt == 3))
    nc.vector.tensor_add(pend["ob"][:, nt, :], ps[:], bias_sb[:])


def _emit_outproj_store(ctx, pend):
    nc = ctx["nc"]
    dst = ctx["out"][pend["s"]].rearrange("(no ni) e -> ni no e", ni=P)
    if ctx["tail"] == "dma_split":
        for nt in range(4):
            nc.sync.dma_start(dst[:, nt, :], pend["ob"][:, nt, :])
    else:
        nc.sync.dma_start(dst, pend["ob"][:])


def _emit_outproj(ctx, pend):
    """Output projection + bias + single batched store for one band."""
    for nt in range(4):
        _emit_outproj_group(ctx, pend, nt)
    _emit_outproj_store(ctx, pend)


def build_kernel(nbands=NBANDS, repeat=1, mm_dtype=MM_DTYPE, ablate="",
                 psum=(2, 2, 2), es_bufs=8, recip="bcast_act", merge_exp=True,
                 tail="dma_split", x_split=False, late_outproj=False,
                 av_merge=False, mul_inplace=False, v_in_attn=False,
                 early_last=False, mul_gpsimd=False, dma_mixed=False):
    import concourse.mybir as mybir
    import concourse.tile as tile
    from concourse import bacc
    from concourse import library_config

    f32 = mybir.dt.float32
    f32r = (mybir.dt.float32r if mm_dtype == "f32r" else mybir.dt.bfloat16)
    Exp = mybir.ActivationFunctionType.Exp
    Ln = mybir.ActivationFunctionType.Ln

    nc = bacc.Bacc("TRN2", target_bir_lowering=False, debug=False,
                   num_devices=NCORES)

    xT = nc.dram_tensor("xT", [nbands, D, NSEQ], f32r, kind="ExternalInput").ap()
    wqkvT = nc.dram_tensor("wqkvT", [D, 3 * D], f32r, kind="ExternalInput").ap()
    woutT = nc.dram_tensor("woutT", [D, D], f32r, kind="ExternalInput").ap()
    biasb = nc.dram_tensor("biasb", [P, D], f32, kind="ExternalInput").ap()
    out = nc.dram_tensor("out", [nbands, NSEQ, D], f32, kind="ExternalOutput").ap()

    nc.gpsimd.load_library(library_config.attn)

    with tile.TileContext(nc) as tc:
        with contextlib.ExitStack() as _stack:
            _p = lambda *a, **kw: _stack.enter_context(tc.tile_pool(*a, **kw))
            wpool = _p(name="weights", bufs=1)
            xpool = _p(name="x", bufs=2)
            qkpool = _p(name="qk", bufs=2)
            vpool = _p(name="v", bufs=2)
            osbpool = _p(name="osb", bufs=2)
            spool = _p(name="es", bufs=es_bufs)
            rccpool = _p(name="rcc", bufs=(2 if recip == "dve" else 1))
            dppool = _p(name="dp", bufs=2)
            rblpool = _p(name="rbl", bufs=1)
            rc2pool = _p(name="rc2", bufs=1)
            lgpool = _p(name="lg", bufs=1)
            rbpool = _p(name="rb", bufs=1)
            otpool = _p(name="ot", bufs=1)
            drampool = _p(name="dram", bufs=2, space="DRAM")
            outpool = _p(name="ob", bufs=1)
            psproj = _p(name="psproj", bufs=psum[0], space="PSUM")
            pss = _p(name="pss", bufs=psum[1], space="PSUM")
            pso = _p(name="pso", bufs=psum[2], space="PSUM")
            # weights: split wq by k-chunk so the first matmuls can start
            # as soon as their chunk lands
            wq_sb = wpool.tile([P, 4, 3 * D], f32r)
            wo_sb = wpool.tile([P, 4, D], f32r)
            bias_sb = wpool.tile([P, D], f32)
            maskA = wpool.tile([1, P], f32)
            maskB = wpool.tile([1, P], f32)
            nc.vector.memset(maskA[:], 0.0)
            nc.vector.memset(maskB[:], 0.0)
            nc.vector.memset(maskA[0:1, 0:DH], 1.0)
            nc.vector.memset(maskB[0:1, DH:P], 1.0)
            wq_r = wqkvT.rearrange("(ko ki) e -> ki ko e", ki=P)
            for kt in range(4):
                nc.sync.dma_start(wq_sb[:, kt, :], wq_r[:, kt, :])
            nc.sync.dma_start(wo_sb[:], woutT.rearrange("(ko ki) e -> ki ko e", ki=P))
            nc.sync.dma_start(bias_sb[:], biasb[:])

            ctx = {
                "nc": nc, "f32": f32, "f32r": f32r, "Exp": Exp, "Ln": Ln,
                "mm_dtype": mm_dtype, "ablate": ablate, "recip": recip,
                "merge_exp": merge_exp, "tail": tail, "av_merge": av_merge,
                "mul_inplace": mul_inplace, "mul_gpsimd": mul_gpsimd,
                "dma_mixed": dma_mixed,
                "wq_sb": wq_sb, "wo_sb": wo_sb, "bias_sb": bias_sb,
                "maskA": maskA, "maskB": maskB,
                "out": out,
                "pools": {
                    "qk": qkpool, "v": vpool, "osb": osbpool, "es": spool,
                    "rcc": rccpool, "rb": rbpool, "rc2": rc2pool,
                    "rbl": rblpool,
                    "ot": otpool, "dram": drampool,
                    "ob": outpool, "psproj": psproj,
                    "pss": pss, "pso": pso,
                },
            }

            def load_x(s):
                xt = xpool.tile([P, 4, NSEQ], f32r, tag="xt")
                xv = xT[s].rearrange("(ko ki) n -> ki ko n", ki=P)
                if x_split:
                    for kt in range(4):
                        nc.sync.dma_start(xt[:, kt, :], xv[:, kt, :])
                else:
                    nc.sync.dma_start(xt[:], xv)
                return xt

            def new_proj_tiles():
                qk_sb = qkpool.tile([P, 8, NSEQ], f32r, tag="qk")
                v_aug = vpool.tile([P, 4, H, DH + 1], f32r, tag="vaug")
                return qk_sb, v_aug

            def proj_thunks(qk_sb, v_aug, xt):
                th = [(lambda e=et: _emit_qkv_group(ctx, qk_sb, xt, e))
                      for et in (0, 4, 1, 5, 2, 6, 3, 7)]
                th += [(lambda n=nt: _emit_v_group(ctx, v_aug, xt, n))
                       for nt in range(4)]
                return th

            def new_attn_state(s, qk_sb, v_aug):
                o_sb = osbpool.tile([P, 4, NSEQ], f32r, tag="osb")
                st = {"s": s, "qk_sb": qk_sb, "v_aug": v_aug, "o_sb": o_sb}
                if recip == "bcast_act":
                    rcc = rccpool.tile([1, 2, 4, NSEQ], f32, tag="rcc")
                    st["rcc"] = rcc
                elif tail == "gather":
                    dp0 = dppool.tile([97, NSEQ], f32, tag="dp0")
                    dp1 = dppool.tile([97, NSEQ], f32, tag="dp1")
                    nc.vector.memset(dp0[:], 1.0)
                    nc.vector.memset(dp1[:], 1.0)
                    st["dp0"], st["dp1"] = dp0, dp1
                else:
                    rcc = rccpool.tile([1, 2, 4, NSEQ], f32, tag="rcc")
                    lg = lgpool.tile([1, 2, 4, NSEQ], f32, tag="lg")
                    st["rcc"], st["lg"] = rcc, lg
                return st

            rep_ctx = (tc.For_i(0, repeat, 1,
                                hint_engines=(mybir.EngineType.PE,
                                              mybir.EngineType.Activation,
                                              mybir.EngineType.DVE))
                       if repeat > 1 else contextlib.nullcontext())
            with rep_ctx:
                # prefetch x one band ahead; tail+outproj lag one band.
                # Schedule per band s: [tail-bounce DMAs for s-1 issued
                # first] qk groups, [normalize mul for s-1 on DVE where
                # its broadcast has landed], v groups, [outproj s-1],
                # attention pairs. Fine-grained interleaving of proj
                # groups into the attention pairs measured SLOWER on HW
                # (cross-engine interlock overhead beats exp-wait fill).
                xt_next = load_x(0)
                pend = None
                for s in range(nbands):
                    xt = xt_next
                    if s + 1 < nbands:
                        xt_next = load_x(s + 1)
                    if pend is not None:
                        _emit_tail_dma(ctx, pend)
                    qk_sb, v_aug = new_proj_tiles()
                    groups = proj_thunks(qk_sb, v_aug, xt)
                    for th in groups[0:8]:
                        th()
                    if pend is not None:
                        _emit_tail_muls(ctx, pend)
                    if not v_in_attn:
                        for th in groups[8:12]:
                            th()
                    if pend is not None and not late_outproj:
                        _emit_outproj(ctx, pend)
                    st = new_attn_state(s, qk_sb, v_aug)
                    if (early_last and s == nbands - 1
                            and recip == "bcast_act" and ablate == ""):
                        dr6 = drampool.tile([1, 2, 4, NSEQ], f32, tag="dr")
                        rb6 = rbpool.tile([P, 4, NSEQ], f32, tag="rb")
                        st["dr"], st["rb"] = dr6, rb6
                    if ctx["ablate"] == "no_attn":
                        nc.vector.tensor_copy(st["o_sb"][:],
                                              st["qk_sb"][:, 0:4, :])
                        if v_in_attn:
                            for th in groups[8:12]:
                                th()
                        if pend is not None and late_outproj:
                            _emit_outproj(ctx, pend)
                    else:
                        for g in range(4):
                            es_list = _emit_pair_s(ctx, st, g)
                            if v_in_attn and g == 0:
                                for th in groups[8:12]:
                                    th()
                            _emit_pair_av(ctx, st, g, es_list)
                            if g == 0 and pend is not None and late_outproj:
                                _emit_outproj(ctx, pend)
                    pend = st
                _emit_tail_dma(ctx, pend)
                _emit_tail_muls(ctx, pend)
                _emit_outproj(ctx, pend)

    nc.compile()
    return nc


def _get_nc():
    global _cached
    if _cached is None:
        _cached = build_kernel()
    return _cached


def make_in_maps(x, x_delta, x_theta, x_alpha, x_beta, x_gamma, x_upper,
                 Wqkv, Wout, bout, mm_dtype=MM_DTYPE):
    if mm_dtype == "f32r":
        cast_dt = np.float32
    else:
        import ml_dtypes
        cast_dt = ml_dtypes.bfloat16
    xs = np.stack([np.asarray(a, dtype=np.float32) for a in
                   (x, x_delta, x_theta, x_alpha, x_beta, x_gamma, x_upper)],
                  axis=0)  # [7, b, n, d]
    xsT = np.ascontiguousarray(xs.transpose(1, 0, 3, 2).astype(cast_dt))
    wqkvT = np.ascontiguousarray(np.asarray(Wqkv, np.float32).T.astype(cast_dt))
    woutT = np.ascontiguousarray(np.asarray(Wout, np.float32).T.astype(cast_dt))
    biasb = np.ascontiguousarray(
        np.broadcast_to(np.asarray(bout, np.float32)[None, :], (P, D)))
    return [
        {"xT": xsT[c], "wqkvT": wqkvT, "woutT": woutT, "biasb": biasb}
        for c in range(NCORES)
    ]


def kernel(x, x_delta, x_theta, x_alpha, x_beta, x_gamma, x_upper,
           Wqkv, Wout, bout):
    from concourse.bass_utils import run_bass_kernel_spmd

    nc = _get_nc()
    in_maps = make_in_maps(x, x_delta, x_theta, x_alpha, x_beta, x_gamma,
                           x_upper, Wqkv, Wout, bout)
    res = run_bass_kernel_spmd(nc, in_maps, core_ids=list(range(NCORES)))
    full = np.empty((NBANDS, NCORES, NSEQ, D), dtype=np.float32)
    for c in range(NCORES):
        full[:, c] = res.results[c]["out"]
    return tuple(full[i] for i in range(NBANDS))
